# revision 16
# baseline (speedup 1.0000x reference)
"""Multi-head attention (B=2, S=2048, D=1024, H=16) on 8 Trainium2 cores.

Sharding: data-parallel over the 2 batches x tensor-parallel over 4 groups
of 4 heads.  Core c handles batch c//4 and heads [4*(c%4) : 4*(c%4)+4]
(columns [256*(c%4) : +256] of Wk/Wv, same rows of Wo).  Each core produces
a partial [S, D] output (its heads' contribution to o @ Wo); the host sums
the 4 partials per batch (and adds bo once).

Per-core dataflow (bf16 matmul operands, fp32 PSUM accumulation):
  qT,kT,vT [D,S] fp32 (host-pre-transposed) are DMA-cast to bf16 on load.
  Projections produce QT,KT [128,2,S] (head-major rows) and V [sk,hd] with
  an extra ones column.  Attention per head in "scores-transposed" layout
  [sk_part, sq_free]: scoresT = KT_j^T @ QT; the causal diagonal adds a
  bf16 -480 lower-triangular tile into PSUM via an identity matmul; exp on
  ScalarE (scale folded in; no max subtraction - scores are O(6));
  UT[65, S] += Vaug_j^T @ expT accumulated in PSUM, row 64 = softmax
  denominators (from the ones column).  Normalization is region-wise
  (512 cols at a time, as soon as that region's last k-block lands):
  sums -> DMA reshape [1,512]->[128,4] -> cheap DVE reciprocal -> DMA back
  -> gpsimd partition_broadcast -> one DVE multiply into oT [d_part, sq].
  Final: out = oT^T @ Wo per 128-row block, fp32 DMA to HBM.
"""

import itertools
import os
from contextlib import ExitStack

import numpy as np

import concourse.bass as bass
import concourse.tile as tile
from concourse import bacc, bass_utils, mybir
from concourse.masks import make_identity

B, S, D, H = 2, 2048, 1024, 16
HD = D // H            # 64
NCORES = 8
HPC = 4                # heads per core
CW = HPC * HD          # 256 weight cols per core
NCH = 4                # sequence chunks of 512
MASKVAL = -480.0       # additive pre-scale causal mask value (exp -> ~e-60)
S_INV = float(1.0 / (np.sqrt(np.float32(HD)) + np.float32(1e-8)))

F32 = mybir.dt.float32
F32R = mybir.dt.float32r
BF16 = mybir.dt.bfloat16


def _build(mode: str, bias_k: bool, bias_v: bool, precision: str = "bf16"):
    """Build + compile the SPMD program.

    mode: 'causal' | 'none' | 'general'
    precision: 'bf16' (everything bf16) or 'mixed' (fp32r projections).
    """
    nc = bacc.Bacc("TRN2", target_bir_lowering=False, debug=False,
                   num_devices=NCORES)
    xdt = BF16 if precision == "bf16" else F32R
    in_dt = F32 if precision == "bf16" else F32R  # dram decl for x/w inputs

    qT_d = nc.dram_tensor("qT", [D, S], in_dt, kind="ExternalInput").ap()
    kT_d = nc.dram_tensor("kT", [D, S], in_dt, kind="ExternalInput").ap()
    vT_d = nc.dram_tensor("vT", [D, S], in_dt, kind="ExternalInput").ap()
    wk_d = nc.dram_tensor("wk", [D, CW], in_dt, kind="ExternalInput").ap()
    wv_d = nc.dram_tensor("wv", [D, CW], in_dt, kind="ExternalInput").ap()
    wo_d = nc.dram_tensor("wo", [CW, D], F32, kind="ExternalInput").ap()
    bk_d = nc.dram_tensor("bk", [1, CW], in_dt, kind="ExternalInput").ap() if bias_k else None
    bv_d = nc.dram_tensor("bv", [1, CW], in_dt, kind="ExternalInput").ap() if bias_v else None
    maskT_d = (nc.dram_tensor("maskT", [S, S], BF16, kind="ExternalInput").ap()
               if mode == "general" else None)
    vones_d = nc.dram_tensor("vones", [128, 16], BF16, kind="ExternalInput").ap()
    ones1_d = (nc.dram_tensor("ones1", [1, 512], xdt, kind="ExternalInput").ap()
               if (bias_k or bias_v) else None)
    out_d = nc.dram_tensor("out", [S, D], F32, kind="ExternalOutput").ap()

    def load(dst, src):
        """DMA load, casting via SWDGE when dtypes differ."""
        if dst.dtype != src.dtype:
            nc.gpsimd.dma_start(dst, src)
        else:
            nc.sync.dma_start(dst, src)

    with tile.TileContext(nc) as tc, ExitStack() as ctx:
        sb1 = ctx.enter_context(tc.tile_pool(name="persist", bufs=1))
        qt_pool = ctx.enter_context(tc.tile_pool(name="qt", bufs=NCH))
        kt_pool = ctx.enter_context(tc.tile_pool(name="kt", bufs=NCH))
        v_pool = ctx.enter_context(tc.tile_pool(name="v", bufs=NCH))
        stage_pool = ctx.enter_context(tc.tile_pool(name="stage", bufs=12 if precision == "bf16" else 8))
        exp_pool = ctx.enter_context(tc.tile_pool(name="exp", bufs=4))
        sums_pool = ctx.enter_context(tc.tile_pool(name="sums", bufs=4))
        srt_pool = ctx.enter_context(tc.tile_pool(name="srt", bufs=4))
        rcb_pool = ctx.enter_context(tc.tile_pool(name="rcb", bufs=4))
        bc_pool = ctx.enter_context(tc.tile_pool(name="bc", bufs=5))
        u_pool = ctx.enter_context(tc.tile_pool(name="u", bufs=4))
        ottmp_pool = ctx.enter_context(tc.tile_pool(name="ottmp", bufs=2))
        outsb_pool = ctx.enter_context(tc.tile_pool(name="outsb", bufs=4))
        win_pool = ctx.enter_context(tc.tile_pool(name="win", bufs=3, space="PSUM"))
        ut_pool = ctx.enter_context(tc.tile_pool(name="ut", bufs=1, space="PSUM"))
        if mode == "general":
            mask_pool = ctx.enter_context(tc.tile_pool(name="mask", bufs=3))

        # ---- constants / weights -------------------------------------
        wk_sb = sb1.tile([128, 8, CW], xdt)
        load(wk_sb[:], wk_d.rearrange("(c p) n -> p c n", p=128))
        wv_sb = sb1.tile([128, 8, CW], xdt)
        load(wv_sb[:], wv_d.rearrange("(c p) n -> p c n", p=128))
        wo_sb = sb1.tile([128, 2, D], BF16)
        load(wo_sb[:], wo_d.rearrange("(m p) n -> p m n", p=128))
        if bias_k:
            bk_sb = sb1.tile([1, CW], xdt)
            load(bk_sb[:], bk_d[:])
        if bias_v:
            bv_sb = sb1.tile([1, CW], xdt)
            load(bv_sb[:], bv_d[:])
        if bias_k or bias_v:
            ones_sb = sb1.tile([1, 512], xdt)
            nc.sync.dma_start(ones_sb[:], ones1_d[:])
        if mode != "none":
            ident = sb1.tile([128, 128], BF16)
            make_identity(nc, ident[:])
        if mode == "causal":
            # dmask[p, f] = MASKVAL where f < p (sq < sk), else 0
            dmask = sb1.tile([128, 128], BF16)
            nc.gpsimd.memset(dmask[:], 0.0)
            nc.gpsimd.affine_select(
                out=dmask[:], in_=dmask[:],
                compare_op=mybir.AluOpType.is_ge,
                fill=MASKVAL, base=0,
                pattern=[[1, 128]], channel_multiplier=-1,
            )

        # V tiles: [128 sk, 4 blk, 4 head, 66] - col 64 is the ones column
        v_tiles = [v_pool.tile([128, 4, HPC, 66], BF16, tag="v", name=f"v{c}")
                   for c in range(NCH)]
        for c in range(NCH):
            nc.sync.dma_start(v_tiles[c][:, :, :, 64:65],
                              vones_d[:].rearrange("p (b h e) -> p b h e", b=4, h=HPC))
        qt_tiles = [qt_pool.tile([128, 2, 512], BF16, tag="qt", name=f"qt{c}")
                    for c in range(NCH)]
        kt_tiles = [kt_pool.tile([128, 2, 512], BF16, tag="kt", name=f"kt{c}")
                    for c in range(NCH)]
        oT_sb = sb1.tile([128, 2, S], BF16)

        copy_engines = itertools.cycle([nc.scalar, nc.vector])

        def ps_copy(dst, src):
            eng = next(copy_engines)
            if eng is nc.scalar:
                nc.scalar.copy(dst, src)
            else:
                nc.vector.tensor_copy(dst, src)

        # ---- phase 1: projections (helpers) --------------------------
        def emit_proj_loads(c):
            sl = bass.ds(c * 512, 512)
            out = []
            for nm, td in (("k", kT_d), ("v", vT_d), ("q", qT_d)):
                halves = []
                for hh in range(2):
                    stg = stage_pool.tile([128, 4, 512], xdt, tag="stage",
                                          name=f"{nm}st{c}_{hh}")
                    load(stg[:], td.rearrange("(cc p) s -> p cc s", p=128)
                         [:, bass.ds(4 * hh, 4), sl])
                    halves.append(stg)
                out.append(halves)
            return out

        def emit_proj_mms(c, stages):
            (kst2, vst2, qst2) = stages
            class _Pair:
                def __init__(self, halves):
                    self.h = halves
                def __getitem__(self, key):
                    p, dc, rest = key[0], key[1], key[2:]
                    return self.h[dc // 4][(p, dc % 4) + rest]
            kst, vst, qst = _Pair(kst2), _Pair(vst2), _Pair(qst2)
            # KT / QT projections (transposed layout, 2 m-halves of 128)
            for ti, (st, dst) in enumerate(((kst, kt_tiles[c]), (qst, qt_tiles[c]))):
                ps = win_pool.tile([128, 1024], F32, tag="win", name=f"psp{c}_{ti}")
                for m in range(2):
                    reg = ps[:, bass.ds(m * 512, 512)]
                    first = True
                    if bias_k:
                        nc.tensor.matmul(reg, bk_sb[0:1, bass.ds(m * 128, 128)],
                                         ones_sb[0:1, :], start=True, stop=False)
                        first = False
                    for dc in range(8):
                        nc.tensor.matmul(
                            reg,
                            wk_sb[:, dc, bass.ds(m * 128, 128)],
                            st[:, dc, :],
                            start=first, stop=(dc == 7))
                        first = False
                ps_copy(dst[:, :, :], ps[:].rearrange("p (m s) -> p m s", m=2))
            # V projection (natural layout)
            psv = win_pool.tile([128, 1024], F32, tag="win", name=f"psv{c}")
            for blk in range(4):
                reg = psv[:, bass.ds(blk * 256, 256)]
                first = True
                if bias_v:
                    nc.tensor.matmul(reg, ones_sb[0:1, 0:128], bv_sb[0:1, :],
                                     start=True, stop=False)
                    first = False
                for dc in range(8):
                    nc.tensor.matmul(
                        reg,
                        vst[:, dc, bass.ds(blk * 128, 128)],
                        wv_sb[:, dc, :],
                        start=first, stop=(dc == 7))
                    first = False
            ps_copy(v_tiles[c][:, :, :, 0:64],
                    psv[:].rearrange("p (b h e) -> p b h e", b=4, h=HPC))

        # ---- phase 2: attention, one (head, sq-half) pass ------------
        full_grid = mode != "causal"

        def attn_half(hl, half):
            m = hl // 2
            p0 = 64 * (hl % 2)
            base = 1024 * half
            regions = (2 * half, 2 * half + 1)
            ut = ut_pool.tile([128, 1024], F32, tag="ut", name=f"ut{hl}_{half}")

            if full_grid:
                steps = list(range(16))
                last_j = {r: 15 for r in regions}
            else:
                steps = list(range(8 * half + 8))
                last_j = {r: 4 * r + 3 for r in regions}

            win_ps = {}
            win_exp = {}

            def subchunks(j):
                """(lo_abs, n) pieces of the window active for k-block j."""
                a0 = base if full_grid else max(128 * j, base)
                out = []
                for s in range(2):
                    lo, hi = base + 512 * s, base + 512 * s + 512
                    if hi <= a0:
                        continue
                    out.append((max(lo, a0), hi - max(lo, a0)))
                return out

            def emit_scores(j):
                ps = win_pool.tile([128, 1024], F32, tag="win", name=f"sc{hl}_{half}_{j}")
                win_ps[j] = ps
                if mode == "general":
                    a0 = base
                    mt = mask_pool.tile([128, 1024], BF16, tag="mask",
                                        name=f"mt{hl}_{half}_{j}")
                    nc.sync.dma_start(
                        mt[:],
                        maskT_d[bass.ds(128 * j, 128), bass.ds(base, 1024)])
                lhsT = kt_tiles[j // 4][p0:p0 + 64, m, bass.ds(128 * (j % 4), 128)]
                for lo_abs, n in subchunks(j):
                    reg = ps[:, bass.ds(lo_abs - base, n)]
                    rhs = qt_tiles[lo_abs // 512][p0:p0 + 64, m,
                                                  bass.ds(lo_abs % 512, n)]
                    diag_here = (mode == "causal") and lo_abs <= 128 * j < lo_abs + n
                    mask_here = (mode == "general")
                    nc.tensor.matmul(reg, lhsT, rhs, start=True,
                                     stop=not (diag_here or mask_here))
                    if diag_here:
                        nc.tensor.matmul(ps[:, bass.ds(128 * j - base, 128)],
                                         ident[:], dmask[:], start=False, stop=True)
                    elif mask_here:
                        nc.tensor.matmul(reg, ident[:],
                                         mt[:, bass.ds(lo_abs - base, n)],
                                         start=False, stop=True)

            def emit_exp(j):
                ps = win_ps[j]
                a0 = base if full_grid else max(128 * j, base)
                off = a0 - base
                et = exp_pool.tile([128, 1024], BF16, tag="exp",
                                   name=f"e{hl}_{half}_{j}")
                win_exp[j] = et
                nc.scalar.activation(et[:, off:1024], ps[:, off:1024],
                                     mybir.ActivationFunctionType.Exp, scale=S_INV)

            def emit_pv(j):
                et = win_exp.pop(j)
                win_ps.pop(j)
                for lo_abs, n in subchunks(j):
                    r = lo_abs // 512
                    nc.tensor.matmul(
                        ut[0:65, bass.ds(lo_abs - base, n)],
                        v_tiles[j // 4][:, j % 4, hl, 0:65],
                        et[:, bass.ds(lo_abs - base, n)],
                        start=(j == 0), stop=(j == last_j[r]))

            if p0 == 0:
                dst = oT_sb[0:64, m, bass.ds(base, 1024)]
                ott = None
            else:
                ott = ottmp_pool.tile([64, 1024], BF16, tag="ottmp",
                                      name=f"ott{hl}_{half}")
                dst = ott[:, :]

            def emit_norm(r):
                """copy U+sums out of PSUM, then recip -> bcast -> multiply."""
                u = u_pool.tile([65, 512], F32, tag="u", name=f"u{hl}_{r}")
                nc.vector.tensor_copy(u[:], ut[0:65, bass.ds(512 * r - base, 512)])
                srt = srt_pool.tile([128, 4], F32, tag="srt", name=f"srt{hl}_{r}")
                nc.sync.dma_start(srt[:], u[64:65, :])
                nc.vector.reciprocal(srt[:], srt[:])
                rcb = rcb_pool.tile([1, 512], F32, tag="rcb", name=f"rcb{hl}_{r}")
                nc.sync.dma_start(rcb[0:1, :], srt[:])
                bc = bc_pool.tile([64, 512], F32, tag="bc", name=f"bc{hl}_{r}")
                nc.gpsimd.partition_broadcast(bc[:], rcb[:], channels=64)
                nc.vector.tensor_mul(
                    dst[:, bass.ds(512 * r - base, 512)],
                    u[0:64, :],
                    bc[:, :])
                if p0:
                    nc.sync.dma_start(
                        oT_sb[64:128, m, bass.ds(512 * r, 512)],
                        ott[:, bass.ds(512 * r - base, 512)])

            LOOKAHEAD = 2
            for i in range(min(LOOKAHEAD, len(steps))):
                emit_scores(steps[i])
            for i, j in enumerate(steps):
                if i + LOOKAHEAD < len(steps):
                    emit_scores(steps[i + LOOKAHEAD])
                emit_exp(j)
                emit_pv(j)
                for r in regions:
                    if j == last_j[r]:
                        emit_norm(r)

        def emit_final(sb):
            ob = outsb_pool.tile([128, D], F32, tag="outsb", name=f"ob{sb}")
            ps = win_pool.tile([128, 1024], F32, tag="win", name=f"pso{sb}")
            for nh in range(2):
                reg = ps[:, bass.ds(nh * 512, 512)]
                for mm_ in range(2):
                    nc.tensor.matmul(
                        reg,
                        oT_sb[:, mm_, bass.ds(sb * 128, 128)],
                        wo_sb[:, mm_, bass.ds(nh * 512, 512)],
                        start=(mm_ == 0), stop=(mm_ == 1))
            ps_copy(ob[:], ps[:])
            nc.sync.dma_start(out_d[bass.ds(sb * 128, 128), :], ob[:])

        # ---- orchestration: overlap proj DMA with attention ----------
        st0 = emit_proj_loads(0)
        emit_proj_mms(0, st0)
        st1 = emit_proj_loads(1)
        emit_proj_mms(1, st1)
        st2 = emit_proj_loads(2)
        st3 = emit_proj_loads(3)
        for hl in range(HPC):
            attn_half(hl, 0)
        emit_proj_mms(2, st2)
        emit_proj_mms(3, st3)
        for sb in range(8):
            emit_final(sb)
        for hl in range(HPC):
            attn_half(hl, 1)
        for sb in range(8, 16):
            emit_final(sb)


    nc.compile()
    return nc


_VONES = None
_ONES1 = np.ones((1, 512), dtype=np.float32)

_CACHE = {}


def _precision():
    return os.environ.get("MHA_PRECISION", "bf16")


def _get_nc(mode, bias_k, bias_v):
    key = (mode, bias_k, bias_v, _precision())
    if key not in _CACHE:
        _CACHE[key] = _build(mode, bias_k, bias_v, _precision())
    return _CACHE[key]


def make_in_maps(q, k, v, mask, Wk, bk, Wv, bv, Wo, bo):
    """Host-side sharding. Returns (mode, bias flags, in_maps)."""
    import ml_dtypes

    global _VONES
    if _VONES is None:
        _VONES = np.ones((128, 16), dtype=ml_dtypes.bfloat16)
    ones1 = (_ONES1 if _precision() != "bf16"
             else _ONES1.astype(ml_dtypes.bfloat16))

    q = np.asarray(q, dtype=np.float32)
    k = np.asarray(k, dtype=np.float32)
    v = np.asarray(v, dtype=np.float32)
    Wk = np.asarray(Wk, dtype=np.float32)
    Wv = np.asarray(Wv, dtype=np.float32)
    Wo = np.asarray(Wo, dtype=np.float32)
    bk = np.asarray(bk, dtype=np.float32).reshape(-1)
    bv = np.asarray(bv, dtype=np.float32).reshape(-1)
    bo = np.asarray(bo, dtype=np.float32).reshape(-1)
    mask2d = np.asarray(mask, dtype=np.float32).reshape(S, S)

    if not mask2d.any():
        mode = "none"
    elif np.array_equal(mask2d, np.triu(np.ones((S, S), np.float32), 1)):
        mode = "causal"
    else:
        mode = "general"
    bias_k, bias_v, bias_o = bool(bk.any()), bool(bv.any()), bool(bo.any())

    qT = [np.ascontiguousarray(q[b].T) for b in range(B)]
    kT = [np.ascontiguousarray(k[b].T) for b in range(B)]
    vT = [np.ascontiguousarray(v[b].T) for b in range(B)]
    if mode == "general":
        # pre-scale so adding before the fused exp scale matches the
        # reference's post-scale add:  (raw + m)*S_INV == raw*S_INV + mask*(-1e9)
        maskT = np.ascontiguousarray(
            (mask2d.T * np.float32(-1e9 / S_INV)).astype(ml_dtypes.bfloat16))

    in_maps = []
    for core in range(NCORES):
        b, g = divmod(core, HPC)
        cs = slice(CW * g, CW * (g + 1))
        im = {
            "qT": qT[b], "kT": kT[b], "vT": vT[b],
            "wk": np.ascontiguousarray(Wk[:, cs]),
            "wv": np.ascontiguousarray(Wv[:, cs]),
            "wo": np.ascontiguousarray(Wo[cs, :]),
        }
        im["vones"] = _VONES
        if bias_k or bias_v:
            im["ones1"] = ones1
        if bias_k:
            im["bk"] = np.ascontiguousarray(bk[cs]).reshape(1, CW)
        if bias_v:
            im["bv"] = np.ascontiguousarray(bv[cs]).reshape(1, CW)
        if mode == "general":
            im["maskT"] = maskT
        in_maps.append(im)
    return mode, (bias_k, bias_v, bias_o), in_maps


def assemble(results, bo=None):
    """Sum per-core partial outputs into the full [B, S, D] output."""
    full = np.zeros((B, S, D), dtype=np.float32)
    for b in range(B):
        acc = results[4 * b]["out"].astype(np.float32)
        for c in range(4 * b + 1, 4 * b + 4):
            acc = acc + results[c]["out"]
        if bo is not None:
            acc = acc + bo
        full[b] = acc
    return full


def kernel(q, k, v, mask, Wk, bk, Wv, bv, Wo, bo):
    mode, (bias_k, bias_v, bias_o), in_maps = make_in_maps(
        q, k, v, mask, Wk, bk, Wv, bv, Wo, bo)
    nc = _get_nc(mode, bias_k, bias_v)
    res = bass_utils.run_bass_kernel_spmd(nc, in_maps, core_ids=list(range(NCORES)))
    bo_arr = np.asarray(bo, dtype=np.float32).reshape(-1) if bias_o else None
    return assemble(res.results, bo_arr)


# revision 17
# speedup vs baseline: 1.1936x; 1.1936x over previous
"""Multi-head attention (B=2, S=2048, D=1024, H=16) on 8 Trainium2 cores.

Sharding: data-parallel over the 2 batches x tensor-parallel over 4 groups
of 4 heads.  Core c handles batch c//4 and heads [4*(c%4) : 4*(c%4)+4]
(columns [256*(c%4) : +256] of Wk/Wv, same rows of Wo).  Each core produces
a partial [S, D] output (its heads' contribution to o @ Wo); the host sums
the 4 partials per batch (and adds bo once).

Per-core dataflow (bf16 matmul operands, fp32 PSUM accumulation):
  qT,kT,vT [D,S] fp32 (host-pre-transposed) are DMA-cast to bf16 on load.
  Projections produce QT,KT [128,2,S] (head-major rows) and V [sk,hd] with
  an extra ones column.  Attention per head in "scores-transposed" layout
  [sk_part, sq_free]: scoresT = KT_j^T @ QT; the causal diagonal adds a
  bf16 -480 lower-triangular tile into PSUM via an identity matmul; exp on
  ScalarE (scale folded in; no max subtraction - scores are O(6));
  UT[65, S] += Vaug_j^T @ expT accumulated in PSUM, row 64 = softmax
  denominators (from the ones column).  Normalization is region-wise
  (512 cols at a time, as soon as that region's last k-block lands):
  sums -> DMA reshape [1,512]->[128,4] -> cheap DVE reciprocal -> DMA back
  -> gpsimd partition_broadcast -> one DVE multiply into oT [d_part, sq].
  Final: out = oT^T @ Wo per 128-row block, fp32 DMA to HBM.
"""

import itertools
import os
from contextlib import ExitStack

import numpy as np

import concourse.bass as bass
import concourse.tile as tile
from concourse import bacc, bass_utils, mybir
from concourse.masks import make_identity

B, S, D, H = 2, 2048, 1024, 16
HD = D // H            # 64
NCORES = 8
HPC = 4                # heads per core
CW = HPC * HD          # 256 weight cols per core
NCH = 4                # sequence chunks of 512
MASKVAL = -480.0       # additive pre-scale causal mask value (exp -> ~e-60)
S_INV = float(1.0 / (np.sqrt(np.float32(HD)) + np.float32(1e-8)))

F32 = mybir.dt.float32
F32R = mybir.dt.float32r
BF16 = mybir.dt.bfloat16


def _build(mode: str, bias_k: bool, bias_v: bool, precision: str = "bf16"):
    """Build + compile the SPMD program.

    mode: 'causal' | 'none' | 'general'
    precision: 'bf16' (everything bf16) or 'mixed' (fp32r projections).
    """
    nc = bacc.Bacc("TRN2", target_bir_lowering=False, debug=False,
                   num_devices=NCORES)
    xdt = BF16 if precision == "bf16" else F32R
    in_dt = F32 if precision == "bf16" else F32R  # dram decl for x/w inputs

    qT_d = nc.dram_tensor("qT", [D, S], in_dt, kind="ExternalInput").ap()
    kT_d = nc.dram_tensor("kT", [D, S], in_dt, kind="ExternalInput").ap()
    vT_d = nc.dram_tensor("vT", [D, S], in_dt, kind="ExternalInput").ap()
    wk_d = nc.dram_tensor("wk", [D, CW], in_dt, kind="ExternalInput").ap()
    wv_d = nc.dram_tensor("wv", [D, CW], in_dt, kind="ExternalInput").ap()
    wo_d = nc.dram_tensor("wo", [CW, D], F32, kind="ExternalInput").ap()
    bk_d = nc.dram_tensor("bk", [1, CW], in_dt, kind="ExternalInput").ap() if bias_k else None
    bv_d = nc.dram_tensor("bv", [1, CW], in_dt, kind="ExternalInput").ap() if bias_v else None
    maskT_d = (nc.dram_tensor("maskT", [S, S], BF16, kind="ExternalInput").ap()
               if mode == "general" else None)
    vones_d = nc.dram_tensor("vones", [128, 16], BF16, kind="ExternalInput").ap()
    ones1_d = (nc.dram_tensor("ones1", [1, 512], xdt, kind="ExternalInput").ap()
               if (bias_k or bias_v) else None)
    out_d = nc.dram_tensor("out", [S, D], F32, kind="ExternalOutput").ap()

    def load(dst, src):
        """DMA load, casting via SWDGE when dtypes differ."""
        if dst.dtype != src.dtype:
            nc.gpsimd.dma_start(dst, src)
        else:
            nc.sync.dma_start(dst, src)

    with tile.TileContext(nc) as tc, ExitStack() as ctx:
        sb1 = ctx.enter_context(tc.tile_pool(name="persist", bufs=1))
        qt_pool = ctx.enter_context(tc.tile_pool(name="qt", bufs=NCH))
        kt_pool = ctx.enter_context(tc.tile_pool(name="kt", bufs=NCH))
        v_pool = ctx.enter_context(tc.tile_pool(name="v", bufs=NCH))
        stage_pool = ctx.enter_context(tc.tile_pool(name="stage", bufs=12 if precision == "bf16" else 8))
        exp_pool = ctx.enter_context(tc.tile_pool(name="exp", bufs=4))
        sums_pool = ctx.enter_context(tc.tile_pool(name="sums", bufs=4))
        srt_pool = ctx.enter_context(tc.tile_pool(name="srt", bufs=4))
        rcb_pool = ctx.enter_context(tc.tile_pool(name="rcb", bufs=4))
        bc_pool = ctx.enter_context(tc.tile_pool(name="bc", bufs=5))
        u_pool = ctx.enter_context(tc.tile_pool(name="u", bufs=4))
        ottmp_pool = ctx.enter_context(tc.tile_pool(name="ottmp", bufs=2))
        outsb_pool = ctx.enter_context(tc.tile_pool(name="outsb", bufs=4))
        sc_pool = ctx.enter_context(tc.tile_pool(name="sc", bufs=6, space="PSUM"))
        ut_pool = ctx.enter_context(tc.tile_pool(name="ut", bufs=1, space="PSUM"))
        if mode == "general":
            mask_pool = ctx.enter_context(tc.tile_pool(name="mask", bufs=3))

        # ---- constants / weights -------------------------------------
        wk_sb = sb1.tile([128, 8, CW], xdt)
        load(wk_sb[:], wk_d.rearrange("(c p) n -> p c n", p=128))
        wv_sb = sb1.tile([128, 8, CW], xdt)
        load(wv_sb[:], wv_d.rearrange("(c p) n -> p c n", p=128))
        wo_sb = sb1.tile([128, 2, D], BF16)
        load(wo_sb[:], wo_d.rearrange("(m p) n -> p m n", p=128))
        if bias_k:
            bk_sb = sb1.tile([1, CW], xdt)
            load(bk_sb[:], bk_d[:])
        if bias_v:
            bv_sb = sb1.tile([1, CW], xdt)
            load(bv_sb[:], bv_d[:])
        if bias_k or bias_v:
            ones_sb = sb1.tile([1, 512], xdt)
            nc.sync.dma_start(ones_sb[:], ones1_d[:])
        if mode != "none":
            ident = sb1.tile([128, 128], BF16)
            make_identity(nc, ident[:])
        if mode == "causal":
            # dmask[p, f] = MASKVAL where f < p (sq < sk), else 0
            dmask = sb1.tile([128, 128], BF16)
            nc.gpsimd.memset(dmask[:], 0.0)
            nc.gpsimd.affine_select(
                out=dmask[:], in_=dmask[:],
                compare_op=mybir.AluOpType.is_ge,
                fill=MASKVAL, base=0,
                pattern=[[1, 128]], channel_multiplier=-1,
            )

        # V tiles: [128 sk, 4 blk, 4 head, 66] - col 64 is the ones column
        v_tiles = [v_pool.tile([128, 4, HPC, 66], BF16, tag="v", name=f"v{c}")
                   for c in range(NCH)]
        for c in range(NCH):
            nc.sync.dma_start(v_tiles[c][:, :, :, 64:65],
                              vones_d[:].rearrange("p (b h e) -> p b h e", b=4, h=HPC))
        qt_tiles = [qt_pool.tile([128, 2, 512], BF16, tag="qt", name=f"qt{c}")
                    for c in range(NCH)]
        kt_tiles = [kt_pool.tile([128, 2, 512], BF16, tag="kt", name=f"kt{c}")
                    for c in range(NCH)]
        oT_sb = sb1.tile([128, 2, S], BF16)

        copy_engines = itertools.cycle([nc.scalar, nc.vector])

        def ps_copy(dst, src):
            eng = next(copy_engines)
            if eng is nc.scalar:
                nc.scalar.copy(dst, src)
            else:
                nc.vector.tensor_copy(dst, src)

        # ---- phase 1: projections (helpers) --------------------------
        def emit_proj_loads(c):
            sl = bass.ds(c * 512, 512)
            out = []
            for nm, td in (("k", kT_d), ("v", vT_d), ("q", qT_d)):
                halves = []
                for hh in range(2):
                    stg = stage_pool.tile([128, 4, 512], xdt, tag="stage",
                                          name=f"{nm}st{c}_{hh}")
                    load(stg[:], td.rearrange("(cc p) s -> p cc s", p=128)
                         [:, bass.ds(4 * hh, 4), sl])
                    halves.append(stg)
                out.append(halves)
            return out

        def emit_proj_mms(c, stages):
            (kst2, vst2, qst2) = stages
            class _Pair:
                def __init__(self, halves):
                    self.h = halves
                def __getitem__(self, key):
                    p, dc, rest = key[0], key[1], key[2:]
                    return self.h[dc // 4][(p, dc % 4) + rest]
            kst, vst, qst = _Pair(kst2), _Pair(vst2), _Pair(qst2)
            # KT / QT projections (transposed layout, 2 m-halves of 128)
            for ti, (st, dst) in enumerate(((kst, kt_tiles[c]), (qst, qt_tiles[c]))):
                for m in range(2):
                    ps = sc_pool.tile([128, 512], F32, tag="sc", name=f"psp{c}_{ti}_{m}")
                    first = True
                    if bias_k:
                        nc.tensor.matmul(ps[:], bk_sb[0:1, bass.ds(m * 128, 128)],
                                         ones_sb[0:1, :], start=True, stop=False)
                        first = False
                    for dc in range(8):
                        nc.tensor.matmul(
                            ps[:],
                            wk_sb[:, dc, bass.ds(m * 128, 128)],
                            st[:, dc, :],
                            start=first, stop=(dc == 7))
                        first = False
                    ps_copy(dst[:, m, :], ps[:])
            # V projection (natural layout)
            for half in range(2):
                psv = sc_pool.tile([128, 512], F32, tag="sc", name=f"psv{c}_{half}")
                for loc in range(2):
                    blk = 2 * half + loc
                    reg = psv[:, bass.ds(loc * 256, 256)]
                    first = True
                    if bias_v:
                        nc.tensor.matmul(reg, ones_sb[0:1, 0:128], bv_sb[0:1, :],
                                         start=True, stop=False)
                        first = False
                    for dc in range(8):
                        nc.tensor.matmul(
                            reg,
                            vst[:, dc, bass.ds(blk * 128, 128)],
                            wv_sb[:, dc, :],
                            start=first, stop=(dc == 7))
                        first = False
                ps_copy(v_tiles[c][:, bass.ds(2 * half, 2), :, 0:64],
                        psv[:].rearrange("p (b h e) -> p b h e", b=2, h=HPC))

        # ---- phase 2: attention, one (head, sq-half) pass ------------
        full_grid = mode != "causal"

        def attn_half(hl, half):
            m = hl // 2
            p0 = 64 * (hl % 2)
            base = 1024 * half
            regions = (2 * half, 2 * half + 1)
            ut = ut_pool.tile([128, 1024], F32, tag="ut", name=f"ut{hl}_{half}")

            if full_grid:
                steps = [(j, r) for j in range(16) for r in regions]
                last_j = {r: 15 for r in regions}
            else:
                steps = [(j, r) for j in range(16) for r in regions if j <= 4 * r + 3]
                last_j = {r: 4 * r + 3 for r in regions}

            win_ps = {}
            win_exp = {}

            def emit_scores(t):
                j, r = t
                ps = sc_pool.tile([128, 512], F32, tag="sc", name=f"sc{hl}_{j}_{r}")
                win_ps[t] = ps
                lo, hi = 512 * r, 512 * r + 512
                nlo = lo if full_grid else max(128 * j, lo)
                n = hi - nlo
                if mode == "general":
                    mt = mask_pool.tile([128, 512], BF16, tag="mask",
                                        name=f"mt{hl}_{j}_{r}")
                    nc.sync.dma_start(
                        mt[:, nlo - lo:],
                        maskT_d[bass.ds(128 * j, 128), bass.ds(nlo, n)])
                reg = ps[:, bass.ds(nlo - lo, n)]
                rhs = qt_tiles[r][p0:p0 + 64, m, bass.ds(nlo % 512, n)]
                lhsT = kt_tiles[j // 4][p0:p0 + 64, m, bass.ds(128 * (j % 4), 128)]
                diag_here = (mode == "causal") and lo <= 128 * j < hi
                mask_here = (mode == "general")
                nc.tensor.matmul(reg, lhsT, rhs, start=True,
                                 stop=not (diag_here or mask_here))
                if diag_here:
                    nc.tensor.matmul(ps[:, bass.ds(128 * j - lo, 128)],
                                     ident[:], dmask[:], start=False, stop=True)
                elif mask_here:
                    nc.tensor.matmul(reg, ident[:], mt[:, bass.ds(nlo - lo, n)],
                                     start=False, stop=True)

            def emit_exp(t):
                j, r = t
                ps = win_ps[t]
                lo = 512 * r
                off = 0 if full_grid else max(128 * j - lo, 0)
                et = exp_pool.tile([128, 512], BF16, tag="exp", name=f"e{hl}_{j}_{r}")
                win_exp[t] = et
                nc.scalar.activation(et[:, off:512], ps[:, off:512],
                                     mybir.ActivationFunctionType.Exp, scale=S_INV)

            def emit_pv(t):
                j, r = t
                et = win_exp.pop(t)
                win_ps.pop(t)
                lo = 512 * r
                off = 0 if full_grid else max(128 * j - lo, 0)
                nc.tensor.matmul(
                    ut[0:65, bass.ds(lo - base + off, 512 - off)],
                    v_tiles[j // 4][:, j % 4, hl, 0:65],
                    et[:, bass.ds(off, 512 - off)],
                    start=(j == 0), stop=(j == last_j[r]))

            if p0 == 0:
                dst = oT_sb[0:64, m, bass.ds(base, 1024)]
                ott = None
            else:
                ott = ottmp_pool.tile([64, 1024], BF16, tag="ottmp",
                                      name=f"ott{hl}_{half}")
                dst = ott[:, :]

            def emit_norm(r):
                """copy U+sums out of PSUM, then recip -> bcast -> multiply."""
                u = u_pool.tile([65, 512], F32, tag="u", name=f"u{hl}_{r}")
                nc.vector.tensor_copy(u[:], ut[0:65, bass.ds(512 * r - base, 512)])
                srt = srt_pool.tile([128, 4], F32, tag="srt", name=f"srt{hl}_{r}")
                nc.sync.dma_start(srt[:], u[64:65, :])
                nc.vector.reciprocal(srt[:], srt[:])
                rcb = rcb_pool.tile([1, 512], F32, tag="rcb", name=f"rcb{hl}_{r}")
                nc.sync.dma_start(rcb[0:1, :], srt[:])
                bc = bc_pool.tile([64, 512], F32, tag="bc", name=f"bc{hl}_{r}")
                nc.gpsimd.partition_broadcast(bc[:], rcb[:], channels=64)
                nc.vector.tensor_mul(
                    dst[:, bass.ds(512 * r - base, 512)],
                    u[0:64, :],
                    bc[:, :])
                if p0:
                    nc.sync.dma_start(
                        oT_sb[64:128, m, bass.ds(512 * r, 512)],
                        ott[:, bass.ds(512 * r - base, 512)])

            LOOKAHEAD = 3
            for i in range(min(LOOKAHEAD, len(steps))):
                emit_scores(steps[i])
            for i, t in enumerate(steps):
                if i + LOOKAHEAD < len(steps):
                    emit_scores(steps[i + LOOKAHEAD])
                emit_exp(t)
                emit_pv(t)
                for r in regions:
                    if t == (last_j[r], r):
                        emit_norm(r)

        def emit_final(sb):
            ob = outsb_pool.tile([128, D], F32, tag="outsb", name=f"ob{sb}")
            for nh in range(2):
                ps = sc_pool.tile([128, 512], F32, tag="sc", name=f"pso{sb}_{nh}")
                for mm_ in range(2):
                    nc.tensor.matmul(
                        ps[:],
                        oT_sb[:, mm_, bass.ds(sb * 128, 128)],
                        wo_sb[:, mm_, bass.ds(nh * 512, 512)],
                        start=(mm_ == 0), stop=(mm_ == 1))
                ps_copy(ob[:, bass.ds(nh * 512, 512)], ps[:])
            nc.sync.dma_start(out_d[bass.ds(sb * 128, 128), :], ob[:])

        # ---- orchestration: overlap proj DMA with attention ----------
        st0 = emit_proj_loads(0)
        emit_proj_mms(0, st0)
        st1 = emit_proj_loads(1)
        emit_proj_mms(1, st1)
        st2 = emit_proj_loads(2)
        st3 = emit_proj_loads(3)
        attn_half(0, 0)
        attn_half(1, 0)
        emit_proj_mms(2, st2)
        attn_half(2, 0)
        attn_half(3, 0)
        emit_proj_mms(3, st3)
        for sb in range(8):
            emit_final(sb)
        for hl in range(HPC):
            attn_half(hl, 1)
        for sb in range(8, 16):
            emit_final(sb)


    nc.compile()
    return nc


_VONES = None
_ONES1 = np.ones((1, 512), dtype=np.float32)

_CACHE = {}


def _precision():
    return os.environ.get("MHA_PRECISION", "bf16")


def _get_nc(mode, bias_k, bias_v):
    key = (mode, bias_k, bias_v, _precision())
    if key not in _CACHE:
        _CACHE[key] = _build(mode, bias_k, bias_v, _precision())
    return _CACHE[key]


def make_in_maps(q, k, v, mask, Wk, bk, Wv, bv, Wo, bo):
    """Host-side sharding. Returns (mode, bias flags, in_maps)."""
    import ml_dtypes

    global _VONES
    if _VONES is None:
        _VONES = np.ones((128, 16), dtype=ml_dtypes.bfloat16)
    ones1 = (_ONES1 if _precision() != "bf16"
             else _ONES1.astype(ml_dtypes.bfloat16))

    q = np.asarray(q, dtype=np.float32)
    k = np.asarray(k, dtype=np.float32)
    v = np.asarray(v, dtype=np.float32)
    Wk = np.asarray(Wk, dtype=np.float32)
    Wv = np.asarray(Wv, dtype=np.float32)
    Wo = np.asarray(Wo, dtype=np.float32)
    bk = np.asarray(bk, dtype=np.float32).reshape(-1)
    bv = np.asarray(bv, dtype=np.float32).reshape(-1)
    bo = np.asarray(bo, dtype=np.float32).reshape(-1)
    mask2d = np.asarray(mask, dtype=np.float32).reshape(S, S)

    if not mask2d.any():
        mode = "none"
    elif np.array_equal(mask2d, np.triu(np.ones((S, S), np.float32), 1)):
        mode = "causal"
    else:
        mode = "general"
    bias_k, bias_v, bias_o = bool(bk.any()), bool(bv.any()), bool(bo.any())

    qT = [np.ascontiguousarray(q[b].T) for b in range(B)]
    kT = [np.ascontiguousarray(k[b].T) for b in range(B)]
    vT = [np.ascontiguousarray(v[b].T) for b in range(B)]
    if mode == "general":
        # pre-scale so adding before the fused exp scale matches the
        # reference's post-scale add:  (raw + m)*S_INV == raw*S_INV + mask*(-1e9)
        maskT = np.ascontiguousarray(
            (mask2d.T * np.float32(-1e9 / S_INV)).astype(ml_dtypes.bfloat16))

    in_maps = []
    for core in range(NCORES):
        b, g = divmod(core, HPC)
        cs = slice(CW * g, CW * (g + 1))
        im = {
            "qT": qT[b], "kT": kT[b], "vT": vT[b],
            "wk": np.ascontiguousarray(Wk[:, cs]),
            "wv": np.ascontiguousarray(Wv[:, cs]),
            "wo": np.ascontiguousarray(Wo[cs, :]),
        }
        im["vones"] = _VONES
        if bias_k or bias_v:
            im["ones1"] = ones1
        if bias_k:
            im["bk"] = np.ascontiguousarray(bk[cs]).reshape(1, CW)
        if bias_v:
            im["bv"] = np.ascontiguousarray(bv[cs]).reshape(1, CW)
        if mode == "general":
            im["maskT"] = maskT
        in_maps.append(im)
    return mode, (bias_k, bias_v, bias_o), in_maps


def assemble(results, bo=None):
    """Sum per-core partial outputs into the full [B, S, D] output."""
    full = np.zeros((B, S, D), dtype=np.float32)
    for b in range(B):
        acc = results[4 * b]["out"].astype(np.float32)
        for c in range(4 * b + 1, 4 * b + 4):
            acc = acc + results[c]["out"]
        if bo is not None:
            acc = acc + bo
        full[b] = acc
    return full


def kernel(q, k, v, mask, Wk, bk, Wv, bv, Wo, bo):
    mode, (bias_k, bias_v, bias_o), in_maps = make_in_maps(
        q, k, v, mask, Wk, bk, Wv, bv, Wo, bo)
    nc = _get_nc(mode, bias_k, bias_v)
    res = bass_utils.run_bass_kernel_spmd(nc, in_maps, core_ids=list(range(NCORES)))
    bo_arr = np.asarray(bo, dtype=np.float32).reshape(-1) if bias_o else None
    return assemble(res.results, bo_arr)


# revision 18
# speedup vs baseline: 1.2611x; 1.0565x over previous
"""Multi-head attention (B=2, S=2048, D=1024, H=16) on 8 Trainium2 cores.

Sharding: data-parallel over the 2 batches x tensor-parallel over 4 groups
of 4 heads.  Core c handles batch c//4 and heads [4*(c%4) : 4*(c%4)+4]
(columns [256*(c%4) : +256] of Wk/Wv, same rows of Wo).  Each core produces
a partial [S, D] output (its heads' contribution to o @ Wo); the host sums
the 4 partials per batch (and adds bo once).

Per-core dataflow (bf16 matmul operands, fp32 PSUM accumulation):
  qT,kT,vT [D,S] fp32 (host-pre-transposed) are DMA-cast to bf16 on load.
  Projections produce QT,KT [128,2,S] (head-major rows) and V [sk,hd] with
  an extra ones column.  Attention per head in "scores-transposed" layout
  [sk_part, sq_free]: scoresT = KT_j^T @ QT; the causal diagonal adds a
  bf16 -480 lower-triangular tile into PSUM via an identity matmul; exp on
  ScalarE (scale folded in; no max subtraction - scores are O(6));
  UT[65, S] += Vaug_j^T @ expT accumulated in PSUM, row 64 = softmax
  denominators (from the ones column).  Normalization is region-wise
  (512 cols at a time, as soon as that region's last k-block lands):
  sums -> DMA reshape [1,512]->[128,4] -> cheap DVE reciprocal -> DMA back
  -> gpsimd partition_broadcast -> one DVE multiply into oT [d_part, sq].
  Final: out = oT^T @ Wo per 128-row block, fp32 DMA to HBM.
"""

import itertools
import os
from contextlib import ExitStack

import numpy as np

import concourse.bass as bass
import concourse.tile as tile
from concourse import bacc, bass_utils, mybir
from concourse.masks import make_identity

B, S, D, H = 2, 2048, 1024, 16
HD = D // H            # 64
NCORES = 8
HPC = 4                # heads per core
CW = HPC * HD          # 256 weight cols per core
NCH = 4                # sequence chunks of 512
MASKVAL = -480.0       # additive pre-scale causal mask value (exp -> ~e-60)
S_INV = float(1.0 / (np.sqrt(np.float32(HD)) + np.float32(1e-8)))

F32 = mybir.dt.float32
F32R = mybir.dt.float32r
BF16 = mybir.dt.bfloat16


def _build(mode: str, bias_k: bool, bias_v: bool, precision: str = "bf16"):
    """Build + compile the SPMD program.

    mode: 'causal' | 'none' | 'general'
    precision: 'bf16' (everything bf16) or 'mixed' (fp32r projections).
    """
    nc = bacc.Bacc("TRN2", target_bir_lowering=False, debug=False,
                   num_devices=NCORES)
    xdt = BF16 if precision == "bf16" else F32R
    in_dt = F32 if precision == "bf16" else F32R  # dram decl for x/w inputs

    qT_d = nc.dram_tensor("qT", [D, S], in_dt, kind="ExternalInput").ap()
    kT_d = nc.dram_tensor("kT", [D, S], in_dt, kind="ExternalInput").ap()
    vT_d = nc.dram_tensor("vT", [D, S], in_dt, kind="ExternalInput").ap()
    wk_d = nc.dram_tensor("wk", [D, CW], in_dt, kind="ExternalInput").ap()
    wv_d = nc.dram_tensor("wv", [D, CW], in_dt, kind="ExternalInput").ap()
    wo_d = nc.dram_tensor("wo", [CW, D], F32, kind="ExternalInput").ap()
    bk_d = nc.dram_tensor("bk", [1, CW], in_dt, kind="ExternalInput").ap() if bias_k else None
    bv_d = nc.dram_tensor("bv", [1, CW], in_dt, kind="ExternalInput").ap() if bias_v else None
    maskT_d = (nc.dram_tensor("maskT", [S, S], BF16, kind="ExternalInput").ap()
               if mode == "general" else None)
    vones_d = nc.dram_tensor("vones", [128, 16], BF16, kind="ExternalInput").ap()
    ones1_d = (nc.dram_tensor("ones1", [1, 512], xdt, kind="ExternalInput").ap()
               if (bias_k or bias_v) else None)
    out_d = nc.dram_tensor("out", [S, D], F32, kind="ExternalOutput").ap()

    def load(dst, src):
        """DMA load, casting via SWDGE when dtypes differ."""
        if dst.dtype != src.dtype:
            nc.gpsimd.dma_start(dst, src)
        else:
            nc.sync.dma_start(dst, src)

    with tile.TileContext(nc) as tc, ExitStack() as ctx:
        sb1 = ctx.enter_context(tc.tile_pool(name="persist", bufs=1))
        qt_pool = ctx.enter_context(tc.tile_pool(name="qt", bufs=NCH))
        kt_pool = ctx.enter_context(tc.tile_pool(name="kt", bufs=NCH))
        v_pool = ctx.enter_context(tc.tile_pool(name="v", bufs=NCH))
        stage_pool = ctx.enter_context(tc.tile_pool(name="stage", bufs=12 if precision == "bf16" else 8))
        exp_pool = ctx.enter_context(tc.tile_pool(name="exp", bufs=4))
        sums_pool = ctx.enter_context(tc.tile_pool(name="sums", bufs=4))
        srt_pool = ctx.enter_context(tc.tile_pool(name="srt", bufs=4))
        rcb_pool = ctx.enter_context(tc.tile_pool(name="rcb", bufs=4))
        bc_pool = ctx.enter_context(tc.tile_pool(name="bc", bufs=5))
        u_pool = ctx.enter_context(tc.tile_pool(name="u", bufs=4))
        ottmp_pool = ctx.enter_context(tc.tile_pool(name="ottmp", bufs=2))
        outsb_pool = ctx.enter_context(tc.tile_pool(name="outsb", bufs=4))
        sc_pool = ctx.enter_context(tc.tile_pool(name="sc", bufs=6, space="PSUM"))
        ut_pool = ctx.enter_context(tc.tile_pool(name="ut", bufs=1, space="PSUM"))
        if mode == "general":
            mask_pool = ctx.enter_context(tc.tile_pool(name="mask", bufs=3))

        # ---- constants / weights -------------------------------------
        wk_sb = sb1.tile([128, 8, CW], xdt)
        load(wk_sb[:], wk_d.rearrange("(c p) n -> p c n", p=128))
        wv_sb = sb1.tile([128, 8, CW], xdt)
        load(wv_sb[:], wv_d.rearrange("(c p) n -> p c n", p=128))
        wo_sb = sb1.tile([128, 2, D], BF16)
        load(wo_sb[:], wo_d.rearrange("(m p) n -> p m n", p=128))
        if bias_k:
            bk_sb = sb1.tile([1, CW], xdt)
            load(bk_sb[:], bk_d[:])
        if bias_v:
            bv_sb = sb1.tile([1, CW], xdt)
            load(bv_sb[:], bv_d[:])
        if bias_k or bias_v:
            ones_sb = sb1.tile([1, 512], xdt)
            nc.sync.dma_start(ones_sb[:], ones1_d[:])
        if mode != "none":
            ident = sb1.tile([128, 128], BF16)
            make_identity(nc, ident[:])
        if mode == "causal":
            # dmask[p, f] = MASKVAL where f < p (sq < sk), else 0
            dmask = sb1.tile([128, 128], BF16)
            nc.gpsimd.memset(dmask[:], 0.0)
            nc.gpsimd.affine_select(
                out=dmask[:], in_=dmask[:],
                compare_op=mybir.AluOpType.is_ge,
                fill=MASKVAL, base=0,
                pattern=[[1, 128]], channel_multiplier=-1,
            )

        # V tiles: [128 sk, 4 blk, 4 head, 66] - col 64 is the ones column
        v_tiles = [v_pool.tile([128, 4, HPC, 66], BF16, tag="v", name=f"v{c}")
                   for c in range(NCH)]
        for c in range(NCH):
            nc.sync.dma_start(v_tiles[c][:, :, :, 64:65],
                              vones_d[:].rearrange("p (b h e) -> p b h e", b=4, h=HPC))
        qt_tiles = [qt_pool.tile([128, 2, 512], BF16, tag="qt", name=f"qt{c}")
                    for c in range(NCH)]
        kt_tiles = [kt_pool.tile([128, 2, 512], BF16, tag="kt", name=f"kt{c}")
                    for c in range(NCH)]
        oT_sb = sb1.tile([128, 2, S], BF16)

        copy_engines = itertools.cycle([nc.scalar, nc.vector])

        def ps_copy(dst, src):
            eng = next(copy_engines)
            if eng is nc.scalar:
                nc.scalar.copy(dst, src)
            else:
                nc.vector.tensor_copy(dst, src)

        # ---- phase 1: projections (helpers) --------------------------
        def emit_proj_loads(c):
            sl = bass.ds(c * 512, 512)
            out = []
            for nm, td in (("k", kT_d), ("v", vT_d), ("q", qT_d)):
                halves = []
                for hh in range(2):
                    stg = stage_pool.tile([128, 4, 512], xdt, tag="stage",
                                          name=f"{nm}st{c}_{hh}")
                    load(stg[:], td.rearrange("(cc p) s -> p cc s", p=128)
                         [:, bass.ds(4 * hh, 4), sl])
                    halves.append(stg)
                out.append(halves)
            return out

        def proj_mm_units(c, stages):
            """Yield once per PSUM accumulation group (small PE work unit)."""
            (kst2, vst2, qst2) = stages
            class _Pair:
                def __init__(self, halves):
                    self.h = halves
                def __getitem__(self, key):
                    p, dc, rest = key[0], key[1], key[2:]
                    return self.h[dc // 4][(p, dc % 4) + rest]
            kst, vst, qst = _Pair(kst2), _Pair(vst2), _Pair(qst2)
            # KT / QT projections (transposed layout, 2 m-halves of 128)
            for ti, (st, dst) in enumerate(((kst, kt_tiles[c]), (qst, qt_tiles[c]))):
                for m in range(2):
                    ps = sc_pool.tile([128, 512], F32, tag="sc", name=f"psp{c}_{ti}_{m}")
                    first = True
                    if bias_k:
                        nc.tensor.matmul(ps[:], bk_sb[0:1, bass.ds(m * 128, 128)],
                                         ones_sb[0:1, :], start=True, stop=False)
                        first = False
                    for dc in range(8):
                        nc.tensor.matmul(
                            ps[:],
                            wk_sb[:, dc, bass.ds(m * 128, 128)],
                            st[:, dc, :],
                            start=first, stop=(dc == 7))
                        first = False
                    ps_copy(dst[:, m, :], ps[:])
                    yield
            # V projection (natural layout)
            for half in range(2):
                psv = sc_pool.tile([128, 512], F32, tag="sc", name=f"psv{c}_{half}")
                for loc in range(2):
                    blk = 2 * half + loc
                    reg = psv[:, bass.ds(loc * 256, 256)]
                    first = True
                    if bias_v:
                        nc.tensor.matmul(reg, ones_sb[0:1, 0:128], bv_sb[0:1, :],
                                         start=True, stop=False)
                        first = False
                    for dc in range(8):
                        nc.tensor.matmul(
                            reg,
                            vst[:, dc, bass.ds(blk * 128, 128)],
                            wv_sb[:, dc, :],
                            start=first, stop=(dc == 7))
                        first = False
                ps_copy(v_tiles[c][:, bass.ds(2 * half, 2), :, 0:64],
                        psv[:].rearrange("p (b h e) -> p b h e", b=2, h=HPC))
                yield

        # ---- phase 2: attention, one (head, sq-half) pass ------------
        full_grid = mode != "causal"

        def attn_half(hl, half):
            m = hl // 2
            p0 = 64 * (hl % 2)
            base = 1024 * half
            regions = (2 * half, 2 * half + 1)
            ut = ut_pool.tile([128, 1024], F32, tag="ut", name=f"ut{hl}_{half}")

            if full_grid:
                steps = [(j, r) for j in range(16) for r in regions]
                last_j = {r: 15 for r in regions}
            else:
                steps = [(j, r) for j in range(16) for r in regions if j <= 4 * r + 3]
                last_j = {r: 4 * r + 3 for r in regions}

            win_ps = {}
            win_exp = {}

            def emit_scores(t):
                j, r = t
                ps = sc_pool.tile([128, 512], F32, tag="sc", name=f"sc{hl}_{j}_{r}")
                win_ps[t] = ps
                lo, hi = 512 * r, 512 * r + 512
                nlo = lo if full_grid else max(128 * j, lo)
                n = hi - nlo
                if mode == "general":
                    mt = mask_pool.tile([128, 512], BF16, tag="mask",
                                        name=f"mt{hl}_{j}_{r}")
                    nc.sync.dma_start(
                        mt[:, nlo - lo:],
                        maskT_d[bass.ds(128 * j, 128), bass.ds(nlo, n)])
                reg = ps[:, bass.ds(nlo - lo, n)]
                rhs = qt_tiles[r][p0:p0 + 64, m, bass.ds(nlo % 512, n)]
                lhsT = kt_tiles[j // 4][p0:p0 + 64, m, bass.ds(128 * (j % 4), 128)]
                diag_here = (mode == "causal") and lo <= 128 * j < hi
                mask_here = (mode == "general")
                nc.tensor.matmul(reg, lhsT, rhs, start=True,
                                 stop=not (diag_here or mask_here))
                if diag_here:
                    nc.tensor.matmul(ps[:, bass.ds(128 * j - lo, 128)],
                                     ident[:], dmask[:], start=False, stop=True)
                elif mask_here:
                    nc.tensor.matmul(reg, ident[:], mt[:, bass.ds(nlo - lo, n)],
                                     start=False, stop=True)

            def emit_exp(t):
                j, r = t
                ps = win_ps[t]
                lo = 512 * r
                off = 0 if full_grid else max(128 * j - lo, 0)
                et = exp_pool.tile([128, 512], BF16, tag="exp", name=f"e{hl}_{j}_{r}")
                win_exp[t] = et
                nc.scalar.activation(et[:, off:512], ps[:, off:512],
                                     mybir.ActivationFunctionType.Exp, scale=S_INV)

            def emit_pv(t):
                j, r = t
                et = win_exp.pop(t)
                win_ps.pop(t)
                lo = 512 * r
                off = 0 if full_grid else max(128 * j - lo, 0)
                nc.tensor.matmul(
                    ut[0:65, bass.ds(lo - base + off, 512 - off)],
                    v_tiles[j // 4][:, j % 4, hl, 0:65],
                    et[:, bass.ds(off, 512 - off)],
                    start=(j == 0), stop=(j == last_j[r]))

            if p0 == 0:
                dst = oT_sb[0:64, m, bass.ds(base, 1024)]
                ott = None
            else:
                ott = ottmp_pool.tile([64, 1024], BF16, tag="ottmp",
                                      name=f"ott{hl}_{half}")
                dst = ott[:, :]

            def emit_norm(r):
                """copy U+sums out of PSUM, then recip -> bcast -> multiply."""
                u = u_pool.tile([65, 512], F32, tag="u", name=f"u{hl}_{r}")
                nc.vector.tensor_copy(u[:], ut[0:65, bass.ds(512 * r - base, 512)])
                srt = srt_pool.tile([128, 4], F32, tag="srt", name=f"srt{hl}_{r}")
                nc.sync.dma_start(srt[:], u[64:65, :])
                nc.vector.reciprocal(srt[:], srt[:])
                rcb = rcb_pool.tile([1, 512], F32, tag="rcb", name=f"rcb{hl}_{r}")
                nc.sync.dma_start(rcb[0:1, :], srt[:])
                bc = bc_pool.tile([64, 512], F32, tag="bc", name=f"bc{hl}_{r}")
                nc.gpsimd.partition_broadcast(bc[:], rcb[:], channels=64)
                nc.vector.tensor_mul(
                    dst[:, bass.ds(512 * r - base, 512)],
                    u[0:64, :],
                    bc[:, :])
                if p0:
                    nc.sync.dma_start(
                        oT_sb[64:128, m, bass.ds(512 * r, 512)],
                        ott[:, bass.ds(512 * r - base, 512)])

            LOOKAHEAD = 3
            for i in range(min(LOOKAHEAD, len(steps))):
                emit_scores(steps[i])
            for i, t in enumerate(steps):
                if i + LOOKAHEAD < len(steps):
                    emit_scores(steps[i + LOOKAHEAD])
                emit_exp(t)
                emit_pv(t)
                for r in regions:
                    if t == (last_j[r], r):
                        emit_norm(r)
                yield

        def emit_final(sb):
            ob = outsb_pool.tile([128, D], F32, tag="outsb", name=f"ob{sb}")
            for nh in range(2):
                ps = sc_pool.tile([128, 512], F32, tag="sc", name=f"pso{sb}_{nh}")
                for mm_ in range(2):
                    nc.tensor.matmul(
                        ps[:],
                        oT_sb[:, mm_, bass.ds(sb * 128, 128)],
                        wo_sb[:, mm_, bass.ds(nh * 512, 512)],
                        start=(mm_ == 0), stop=(mm_ == 1))
                ps_copy(ob[:, bass.ds(nh * 512, 512)], ps[:])
            nc.sync.dma_start(out_d[bass.ds(sb * 128, 128), :], ob[:])

        def drain(gen):
            for _ in gen:
                pass

        def weave(step_gen, unit_gen, steps_per_unit):
            """Emit attention steps, inserting one PE-heavy unit every N."""
            i = 0
            for _ in step_gen:
                i += 1
                if i % steps_per_unit == 0:
                    next(unit_gen, None)
            for _ in unit_gen:
                pass

        def chain(*gens):
            for g in gens:
                yield from g

        # ---- orchestration: overlap proj DMA with attention ----------
        st0 = emit_proj_loads(0)
        drain(proj_mm_units(0, st0))
        st1 = emit_proj_loads(1)
        drain(proj_mm_units(1, st1))
        st2 = emit_proj_loads(2)
        st3 = emit_proj_loads(3)
        half0s = chain(*[attn_half(hl, 0) for hl in range(HPC)])
        proj23 = chain(proj_mm_units(2, st2), proj_mm_units(3, st3))
        weave(half0s, proj23, 4)

        def final_units(lo, hi):
            for sb in range(lo, hi):
                emit_final(sb)
                yield

        half1s = chain(*[attn_half(hl, 1) for hl in range(HPC)])
        weave(half1s, final_units(0, 8), 12)
        for sb in range(8, 16):
            emit_final(sb)


    nc.compile()
    return nc


_VONES = None
_ONES1 = np.ones((1, 512), dtype=np.float32)

_CACHE = {}


def _precision():
    return os.environ.get("MHA_PRECISION", "bf16")


def _get_nc(mode, bias_k, bias_v):
    key = (mode, bias_k, bias_v, _precision())
    if key not in _CACHE:
        _CACHE[key] = _build(mode, bias_k, bias_v, _precision())
    return _CACHE[key]


def make_in_maps(q, k, v, mask, Wk, bk, Wv, bv, Wo, bo):
    """Host-side sharding. Returns (mode, bias flags, in_maps)."""
    import ml_dtypes

    global _VONES
    if _VONES is None:
        _VONES = np.ones((128, 16), dtype=ml_dtypes.bfloat16)
    ones1 = (_ONES1 if _precision() != "bf16"
             else _ONES1.astype(ml_dtypes.bfloat16))

    q = np.asarray(q, dtype=np.float32)
    k = np.asarray(k, dtype=np.float32)
    v = np.asarray(v, dtype=np.float32)
    Wk = np.asarray(Wk, dtype=np.float32)
    Wv = np.asarray(Wv, dtype=np.float32)
    Wo = np.asarray(Wo, dtype=np.float32)
    bk = np.asarray(bk, dtype=np.float32).reshape(-1)
    bv = np.asarray(bv, dtype=np.float32).reshape(-1)
    bo = np.asarray(bo, dtype=np.float32).reshape(-1)
    mask2d = np.asarray(mask, dtype=np.float32).reshape(S, S)

    if not mask2d.any():
        mode = "none"
    elif np.array_equal(mask2d, np.triu(np.ones((S, S), np.float32), 1)):
        mode = "causal"
    else:
        mode = "general"
    bias_k, bias_v, bias_o = bool(bk.any()), bool(bv.any()), bool(bo.any())

    qT = [np.ascontiguousarray(q[b].T) for b in range(B)]
    kT = [np.ascontiguousarray(k[b].T) for b in range(B)]
    vT = [np.ascontiguousarray(v[b].T) for b in range(B)]
    if mode == "general":
        # pre-scale so adding before the fused exp scale matches the
        # reference's post-scale add:  (raw + m)*S_INV == raw*S_INV + mask*(-1e9)
        maskT = np.ascontiguousarray(
            (mask2d.T * np.float32(-1e9 / S_INV)).astype(ml_dtypes.bfloat16))

    in_maps = []
    for core in range(NCORES):
        b, g = divmod(core, HPC)
        cs = slice(CW * g, CW * (g + 1))
        im = {
            "qT": qT[b], "kT": kT[b], "vT": vT[b],
            "wk": np.ascontiguousarray(Wk[:, cs]),
            "wv": np.ascontiguousarray(Wv[:, cs]),
            "wo": np.ascontiguousarray(Wo[cs, :]),
        }
        im["vones"] = _VONES
        if bias_k or bias_v:
            im["ones1"] = ones1
        if bias_k:
            im["bk"] = np.ascontiguousarray(bk[cs]).reshape(1, CW)
        if bias_v:
            im["bv"] = np.ascontiguousarray(bv[cs]).reshape(1, CW)
        if mode == "general":
            im["maskT"] = maskT
        in_maps.append(im)
    return mode, (bias_k, bias_v, bias_o), in_maps


def assemble(results, bo=None):
    """Sum per-core partial outputs into the full [B, S, D] output."""
    full = np.zeros((B, S, D), dtype=np.float32)
    for b in range(B):
        acc = results[4 * b]["out"].astype(np.float32)
        for c in range(4 * b + 1, 4 * b + 4):
            acc = acc + results[c]["out"]
        if bo is not None:
            acc = acc + bo
        full[b] = acc
    return full


def kernel(q, k, v, mask, Wk, bk, Wv, bv, Wo, bo):
    mode, (bias_k, bias_v, bias_o), in_maps = make_in_maps(
        q, k, v, mask, Wk, bk, Wv, bv, Wo, bo)
    nc = _get_nc(mode, bias_k, bias_v)
    res = bass_utils.run_bass_kernel_spmd(nc, in_maps, core_ids=list(range(NCORES)))
    bo_arr = np.asarray(bo, dtype=np.float32).reshape(-1) if bias_o else None
    return assemble(res.results, bo_arr)
